# revision 22
# baseline (speedup 1.0000x reference)
"""CrystalGraphAttention Trainium2 kernel (v2).

Data-parallel over batch: core b handles batch b (B=8, 8 cores).
Per-core algorithm (transposed layouts, [feature, node]):
  xT = x^T                                  (PE transpose)
  qT = (Wq/8)^T xT, kT = Wk^T xT            (fp32r; head pairs row-packed)
  vaug[tc] = (x Wv)[tc-chunk]               (bf16, plain [128,512] chunks)
  mTn[tc]  = (1 - edge_mask^T) chunk, bf16  (host-prepped, exact 0/1)
  logitsT[t,s] = kT_h^T qT_h + (-1e9 I) @ mTn   (mask folded into the logits
       PSUM by a matmul-accumulate; unmasked entries get exactly +0)
  ex = exp(logits * dw_t)  (ACT, per-partition dw scale; masked -> exp(-1e5)=0)
  po_pair[0:64] += v_he^T ex_he ; po_pair[64:128] += v_ho^T ex_ho (col-packed)
  pd[32j] += ones^T ex_h   (4-way col-packed denominators)
  oT = po / den            (recip + gpsimd partition broadcast)
  out = oT^T Wo + bo       (fp32r, bias via ones-row K=1 accumulation)
"""
import sys

if '/opt/trn_rl_repo' not in sys.path:
    sys.path.insert(0, '/opt/trn_rl_repo')

import os

import numpy as np

B, N, D = 8, 1024, 256
H, DK, DV = 8, 64, 64
NCORES = 8

# Schraudolph exp: exp(x) ~= bitcast_f32(int32(A*x + BC))
A_SCH = float(2.0 ** 23 / np.log(2.0))
B_SCH = float((127.0 - 0.044) * 2.0 ** 23)

_COMPILED = {}


def _build():
    import concourse.bass as bass
    import concourse.mybir as mybir
    import concourse.tile as tile
    from concourse import bacc
    from concourse.masks import make_identity

    f32 = mybir.dt.float32
    f32r = mybir.dt.float32r
    bf16 = mybir.dt.bfloat16
    i32 = mybir.dt.int32
    MULT = mybir.AluOpType.mult
    ADD = mybir.AluOpType.add
    EXP = mybir.ActivationFunctionType.Exp

    n_sch = int(os.environ.get("KSCH", "0"))  # of 32 a==1 pair-tiles

    nc = bacc.Bacc(None, target_bir_lowering=False)

    x_d = nc.dram_tensor("x", [N, D], f32, kind="ExternalInput")
    m_d = nc.dram_tensor("m", [N, N], bf16, kind="ExternalInput")
    dwc_d = nc.dram_tensor("dwcol", [128, 8], f32, kind="ExternalInput")
    wq_d = nc.dram_tensor("wq", [D, H * DK], f32, kind="ExternalInput")
    wk_d = nc.dram_tensor("wk", [D, H * DK], f32, kind="ExternalInput")
    wv_d = nc.dram_tensor("wv", [D, H * DV], f32, kind="ExternalInput")
    wo_d = nc.dram_tensor("wo", [H * DV, D], f32, kind="ExternalInput")
    bo_d = nc.dram_tensor("bo", [1, D], f32, kind="ExternalInput")
    out_d = nc.dram_tensor("out", [N, D], f32, kind="ExternalOutput")

    with tile.TileContext(nc) as tc:
        with tc.tile_pool(name="const", bufs=1) as cst, \
             tc.tile_pool(name="big", bufs=1) as big, \
             tc.tile_pool(name="xst", bufs=2) as xpool, \
             tc.tile_pool(name="wst", bufs=2) as wpool, \
             tc.tile_pool(name="exq", bufs=3) as expool, \
             tc.tile_pool(name="nrm", bufs=4) as npool, \
             tc.tile_pool(name="outp", bufs=3) as opool, \
             tc.tile_pool(name="psb", bufs=2, space="PSUM") as ps_big, \
             tc.tile_pool(name="pspo", bufs=2, space="PSUM") as ps_po, \
             tc.tile_pool(name="pspd", bufs=1, space="PSUM") as ps_pd, \
             tc.tile_pool(name="psot", bufs=1, space="PSUM") as ps_out:

            # ---- constants / tiny inputs ----
            ident = cst.tile([128, 128], f32)
            make_identity(nc, ident)
            idn_f = cst.tile([128, 128], f32)
            nc.vector.tensor_scalar_mul(idn_f, ident, -1.0e9)
            idn = cst.tile([128, 128], bf16)
            nc.vector.tensor_copy(idn, idn_f)

            ones_f = cst.tile([1, 128], f32)
            nc.vector.memset(ones_f, 1.0)
            ones_r = cst.tile([1, 128], f32r)
            nc.vector.tensor_copy(ones_r, ones_f)
            ones_c = cst.tile([128, 1], f32)
            nc.vector.memset(ones_c, 1.0)
            ones_bf = cst.tile([128, 1], bf16)
            nc.vector.tensor_copy(ones_bf, ones_c)

            dwc = cst.tile([128, 8], f32)
            nc.scalar.dma_start(dwc, dwc_d[:, :])

            # ---- bulk input DMAs: x first (scalar queue), mask on sync ----
            xsb = big.tile([128, 8, D], f32, name="xsb")
            nc.scalar.dma_start(xsb, x_d.rearrange("(nch p) j -> p nch j", p=128))

            mTn = [big.tile([128, N], bf16, name=f"mTn{t8}") for t8 in range(8)]
            for t8 in range(8):
                nc.sync.dma_start(mTn[t8], m_d[t8 * 128:(t8 + 1) * 128, :])

            # ---- weights (scalar HWDGE queue) -> fp32r (q pre-scaled 1/8) ----
            def load_w(dram, scale):
                st = wpool.tile([128, 1024], f32, tag="wst")
                nc.scalar.dma_start(
                    st.rearrange("p (kd c) -> p kd c", kd=2),
                    dram.rearrange("(kd p) c -> p kd c", p=128))
                r = big.tile([128, 1024], f32r, name=dram.name + "_r")
                if scale is None:
                    nc.vector.tensor_copy(r, st)
                else:
                    nc.vector.tensor_scalar_mul(r, st, scale)
                return r

            wq_r = load_w(wq_d, 0.125)
            wk_r = load_w(wk_d, None)
            wv_r = load_w(wv_d, None)
            wo_st = wpool.tile([128, 1024], f32, tag="wst")
            nc.scalar.dma_start(
                wo_st.rearrange("p (cc c) -> p cc c", cc=4),
                wo_d.rearrange("(cc p) c -> p cc c", p=128))
            wo_r = big.tile([128, 1024], f32r)
            nc.vector.tensor_copy(wo_r, wo_st)
            bo_f = cst.tile([1, 256], f32)
            nc.scalar.dma_start(bo_f, bo_d[:, :])
            bo_r = cst.tile([1, 256], f32r)
            nc.vector.tensor_copy(bo_r, bo_f)

            # ---- xT via PE transpose ----
            xT = big.tile([128, 2 * N], f32r)  # [p=d%128, kd*1024 + n]
            for g in range(2):
                pst = ps_big.tile([128, 1024], f32, tag="ps")
                for i in range(4):
                    nch = g * 4 + i
                    nc.tensor.transpose(pst[:, i * 128:(i + 1) * 128],
                                        xsb[:, nch, 0:128], ident)
                    nc.tensor.transpose(pst[:, 512 + i * 128:512 + (i + 1) * 128],
                                        xsb[:, nch, 128:256], ident)
                nc.vector.tensor_copy(
                    xT.rearrange("p (kd g n) -> p kd g n", kd=2, g=2)[:, :, g, :],
                    pst.rearrange("p (kd n) -> p kd n", kd=2))

            # ---- qT, kT (raw; dw applied at exp time) ----
            qT = big.tile([128, 4 * N], f32r)  # [dk + 64*(h%2), (h//2)*1024 + n]
            kT = big.tile([128, 4 * N], f32r)
            for c4 in range(4):
                for nt in range(2):
                    psqk = ps_big.tile([128, 1024], f32, tag="ps")
                    for kd in range(2):
                        nc.tensor.matmul(
                            psqk[:, 0:512],
                            wq_r[:, kd * 512 + c4 * 128:kd * 512 + (c4 + 1) * 128],
                            xT[:, kd * N + nt * 512:kd * N + nt * 512 + 512],
                            start=(kd == 0), stop=(kd == 1))
                    for kd in range(2):
                        nc.tensor.matmul(
                            psqk[:, 512:1024],
                            wk_r[:, kd * 512 + c4 * 128:kd * 512 + (c4 + 1) * 128],
                            xT[:, kd * N + nt * 512:kd * N + nt * 512 + 512],
                            start=(kd == 0), stop=(kd == 1))
                    off = c4 * N + nt * 512
                    nc.vector.tensor_copy(qT[:, off:off + 512], psqk[:, 0:512])
                    nc.vector.tensor_copy(kT[:, off:off + 512], psqk[:, 512:1024])

            # ---- v chunks (bf16) ----
            vaug = [big.tile([128, 512], bf16, name=f"v{t8}") for t8 in range(8)]
            for t8 in range(8):
                psv = ps_big.tile([128, 512], f32, tag="ps")
                for kd in range(2):
                    nc.tensor.matmul(
                        psv, xT[:, kd * N + t8 * 128:kd * N + (t8 + 1) * 128],
                        wv_r[:, kd * 512:(kd + 1) * 512],
                        start=(kd == 0), stop=(kd == 1))
                nc.vector.tensor_copy(vaug[t8], psv)

            # ---- attention ----
            oT = [big.tile([128, 4 * 512], f32r, name=f"oT{st}") for st in range(2)]
            for st in range(2):
                for half in range(2):
                    po = [ps_po.tile([128, 512], f32, tag="po",
                                     name=f"po_{st}_{half}_{a}") for a in range(2)]
                    pd = ps_pd.tile([128, 512], f32, tag="pd")
                    for t8 in range(8):
                        first, last = (t8 == 0), (t8 == 7)
                        exs = []
                        for a in range(2):
                            p_idx = half * 2 + a
                            co = p_idx * N
                            ls = ps_big.tile([128, 1024], f32, tag="ps",
                                             name=f"ls_{st}_{half}_{t8}_{a}")
                            for e in range(2):
                                nc.tensor.matmul(
                                    ls[:, e * 512:(e + 1) * 512], idn,
                                    mTn[t8][:, st * 512:st * 512 + 512],
                                    start=True, stop=False)
                            for e in range(2):
                                nc.tensor.matmul(
                                    ls[:, e * 512:(e + 1) * 512],
                                    kT[e * 64:(e + 1) * 64,
                                       co + t8 * 128:co + (t8 + 1) * 128],
                                    qT[e * 64:(e + 1) * 64,
                                       co + st * 512:co + st * 512 + 512],
                                    start=False, stop=True)
                            ex = expool.tile([128, 1024], bf16, tag="exq")
                            nc.scalar.activation(ex, ls, EXP,
                                                 scale=dwc[:, t8:t8 + 1])
                            exs.append(ex)
                            for e in range(2):
                                h = 2 * p_idx + e
                                nc.tensor.matmul(
                                    po[a][e * 64:(e + 1) * 64, :],
                                    vaug[t8][:, h * 64:(h + 1) * 64],
                                    ex[:, e * 512:(e + 1) * 512],
                                    start=first, stop=last)
                        # 4-way col-packed denominators
                        for a in range(2):
                            for e in range(2):
                                j = 2 * a + e
                                nc.tensor.matmul(
                                    pd[32 * j:32 * j + 1, :], ones_bf,
                                    exs[a][:, e * 512:(e + 1) * 512],
                                    start=first, stop=last,
                                    tile_position=(0, 32 * j))
                    # normalize this half's 4 heads (one folded reciprocal)
                    rden = npool.tile([128, 512], f32, tag="dsb")
                    nc.vector.reciprocal_approx_fast(rden, pd)
                    for a in range(2):
                        for e in range(2):
                            j = 2 * a + e
                            p_idx = half * 2 + a
                            p0 = 64 * e
                            rr = npool.tile([1, 512], f32, tag="rr")
                            nc.sync.dma_start(rr, rden[32 * j:32 * j + 1, :])
                            rb = npool.tile([128, 512], f32, tag="rb")
                            nc.gpsimd.partition_broadcast(rb, rr)
                            nc.vector.tensor_tensor(
                                oT[st][p0:p0 + 64, p_idx * 512:(p_idx + 1) * 512],
                                po[a][p0:p0 + 64, :], rb[p0:p0 + 64, :], MULT)
                # ---- output projection for this st's 4 row-chunks ----
                for s4 in range(4):
                    sc = st * 4 + s4
                    psp = ps_out.tile([128, 256], f32, tag="psp")
                    for cc in range(4):
                        nc.tensor.matmul(
                            psp, oT[st][:, cc * 512 + s4 * 128:cc * 512 + (s4 + 1) * 128],
                            wo_r[:, cc * 256:(cc + 1) * 256],
                            start=(cc == 0), stop=False)
                    nc.tensor.matmul(psp, ones_r[0:1, :], bo_r[0:1, :],
                                     start=False, stop=True)
                    ot = opool.tile([128, 256], f32, tag="outp")
                    nc.vector.tensor_copy(ot, psp)
                    nc.sync.dma_start(out_d[sc * 128:(sc + 1) * 128, :], ot)

    nc.compile()
    return nc


def _get_compiled():
    if 'nc' not in _COMPILED:
        _COMPILED['nc'] = _build()
    return _COMPILED['nc']


def _shard(inputs):
    import ml_dtypes
    x = np.ascontiguousarray(inputs['node_features'], dtype=np.float32)
    em = np.ascontiguousarray(inputs['edge_mask'], dtype=np.float32)
    dw = np.ascontiguousarray(inputs['distance_weights'], dtype=np.float32)
    wq = np.ascontiguousarray(inputs['Wq'], dtype=np.float32)
    wk = np.ascontiguousarray(inputs['Wk'], dtype=np.float32)
    wv = np.ascontiguousarray(inputs['Wv'], dtype=np.float32)
    wo = np.ascontiguousarray(inputs['Wo'], dtype=np.float32)
    bo = np.ascontiguousarray(inputs['bo'], dtype=np.float32).reshape(1, D)
    maps = []
    for b in range(NCORES):
        m_bf = np.ascontiguousarray(1.0 - em[b, 0].T).astype(ml_dtypes.bfloat16)
        maps.append({
            "x": x[b],
            "m": m_bf,
            "dwcol": np.ascontiguousarray(dw[b].reshape(8, 128).T),
            "wq": wq, "wk": wk, "wv": wv, "wo": wo, "bo": bo,
        })
    return maps


def run_sharded(inputs, **kwargs):
    from concourse.bass_utils import run_bass_kernel_spmd
    nc = _get_compiled()
    maps = _shard(inputs)
    res = run_bass_kernel_spmd(nc, maps, core_ids=list(range(NCORES)), **kwargs)
    out = np.stack([res.results[b]["out"] for b in range(NCORES)], axis=0)
    return out, res


def kernel(**inputs) -> np.ndarray:
    out, _ = run_sharded(inputs)
    return out


# revision 23
# speedup vs baseline: 1.0834x; 1.0834x over previous
"""CrystalGraphAttention Trainium2 kernel (v2).

Data-parallel over batch: core b handles batch b (B=8, 8 cores).
Per-core algorithm (transposed layouts, [feature, node]):
  xT = x^T                                  (PE transpose)
  qT = (Wq/8)^T xT, kT = Wk^T xT            (fp32r; head pairs row-packed)
  vaug[tc] = (x Wv)[tc-chunk]               (bf16, plain [128,512] chunks)
  mTn[tc]  = (1 - edge_mask^T) chunk, bf16  (host-prepped, exact 0/1)
  logitsT[t,s] = kT_h^T qT_h + (-1e9 I) @ mTn   (mask folded into the logits
       PSUM by a matmul-accumulate; unmasked entries get exactly +0)
  ex = exp(logits * dw_t)  (ACT, per-partition dw scale; masked -> exp(-1e5)=0)
  po_pair[0:64] += v_he^T ex_he ; po_pair[64:128] += v_ho^T ex_ho (col-packed)
  pd[32j] += ones^T ex_h   (4-way col-packed denominators)
  oT = po / den            (recip + gpsimd partition broadcast)
  out = oT^T Wo + bo       (fp32r, bias via ones-row K=1 accumulation)
"""
import sys

if '/opt/trn_rl_repo' not in sys.path:
    sys.path.insert(0, '/opt/trn_rl_repo')

import os

import numpy as np

B, N, D = 8, 1024, 256
H, DK, DV = 8, 64, 64
NCORES = 8

# Schraudolph exp: exp(x) ~= bitcast_f32(int32(A*x + BC))
A_SCH = float(2.0 ** 23 / np.log(2.0))
B_SCH = float((127.0 - 0.044) * 2.0 ** 23)

_COMPILED = {}


def _build():
    import concourse.bass as bass
    import concourse.mybir as mybir
    import concourse.tile as tile
    from concourse import bacc
    from concourse.masks import make_identity

    f32 = mybir.dt.float32
    f32r = mybir.dt.float32r
    bf16 = mybir.dt.bfloat16
    i32 = mybir.dt.int32
    MULT = mybir.AluOpType.mult
    ADD = mybir.AluOpType.add
    EXP = mybir.ActivationFunctionType.Exp

    n_sch = int(os.environ.get("KSCH", "0"))  # of 32 a==1 pair-tiles

    nc = bacc.Bacc(None, target_bir_lowering=False)

    x_d = nc.dram_tensor("x", [N, D], f32, kind="ExternalInput")
    m_d = nc.dram_tensor("m", [N, N], bf16, kind="ExternalInput")
    dwc_d = nc.dram_tensor("dwcol", [128, 8], f32, kind="ExternalInput")
    wq_d = nc.dram_tensor("wq", [D, H * DK], f32, kind="ExternalInput")
    wk_d = nc.dram_tensor("wk", [D, H * DK], f32, kind="ExternalInput")
    wv_d = nc.dram_tensor("wv", [D, H * DV], f32, kind="ExternalInput")
    wo_d = nc.dram_tensor("wo", [H * DV, D], f32, kind="ExternalInput")
    bo_d = nc.dram_tensor("bo", [1, D], f32, kind="ExternalInput")
    out_d = nc.dram_tensor("out", [N, D], f32, kind="ExternalOutput")

    with tile.TileContext(nc) as tc:
        with tc.tile_pool(name="const", bufs=1) as cst, \
             tc.tile_pool(name="big", bufs=1) as big, \
             tc.tile_pool(name="xst", bufs=2) as xpool, \
             tc.tile_pool(name="wst", bufs=2) as wpool, \
             tc.tile_pool(name="exq", bufs=3) as expool, \
             tc.tile_pool(name="nrm", bufs=4) as npool, \
             tc.tile_pool(name="outp", bufs=3) as opool, \
             tc.tile_pool(name="psb", bufs=2, space="PSUM") as ps_big, \
             tc.tile_pool(name="pspo", bufs=2, space="PSUM") as ps_po, \
             tc.tile_pool(name="pspd", bufs=1, space="PSUM") as ps_pd, \
             tc.tile_pool(name="psot", bufs=1, space="PSUM") as ps_out:

            # ---- constants / tiny inputs ----
            ident = cst.tile([128, 128], f32)
            make_identity(nc, ident)
            idn_f = cst.tile([128, 128], f32)
            nc.vector.tensor_scalar_mul(idn_f, ident, -1.0e9)
            idn = cst.tile([128, 128], bf16)
            nc.vector.tensor_copy(idn, idn_f)

            ones_f = cst.tile([1, 128], f32)
            nc.vector.memset(ones_f, 1.0)
            ones_r = cst.tile([1, 128], bf16)
            nc.vector.tensor_copy(ones_r, ones_f)
            ones_c = cst.tile([128, 1], f32)
            nc.vector.memset(ones_c, 1.0)
            ones_bf = cst.tile([128, 1], bf16)
            nc.vector.tensor_copy(ones_bf, ones_c)

            dwc = cst.tile([128, 8], f32)
            nc.scalar.dma_start(dwc, dwc_d[:, :])

            # ---- bulk input DMAs: x first (scalar queue), mask on sync ----
            xsb = big.tile([128, 8, D], f32, name="xsb")
            nc.scalar.dma_start(xsb, x_d.rearrange("(nch p) j -> p nch j", p=128))

            mTn = [big.tile([128, N], bf16, name=f"mTn{t8}") for t8 in range(8)]
            for t8 in range(8):
                nc.sync.dma_start(mTn[t8], m_d[t8 * 128:(t8 + 1) * 128, :])

            # ---- weights (scalar HWDGE queue) -> fp32r (q pre-scaled 1/8) ----
            def load_w(dram, scale):
                st = wpool.tile([128, 1024], f32, tag="wst")
                nc.scalar.dma_start(
                    st.rearrange("p (kd c) -> p kd c", kd=2),
                    dram.rearrange("(kd p) c -> p kd c", p=128))
                r = big.tile([128, 1024], bf16, name=dram.name + "_r")
                if scale is None:
                    nc.vector.tensor_copy(r, st)
                else:
                    nc.vector.tensor_scalar_mul(r, st, scale)
                return r

            wq_r = load_w(wq_d, 0.125)
            wk_r = load_w(wk_d, None)
            wv_r = load_w(wv_d, None)
            wo_st = wpool.tile([128, 1024], f32, tag="wst")
            nc.scalar.dma_start(
                wo_st.rearrange("p (cc c) -> p cc c", cc=4),
                wo_d.rearrange("(cc p) c -> p cc c", p=128))
            wo_r = big.tile([128, 1024], bf16)
            nc.vector.tensor_copy(wo_r, wo_st)
            bo_f = cst.tile([1, 256], f32)
            nc.scalar.dma_start(bo_f, bo_d[:, :])
            bo_r = cst.tile([1, 256], bf16)
            nc.vector.tensor_copy(bo_r, bo_f)

            # ---- xT via PE transpose ----
            xT = big.tile([128, 2 * N], bf16)  # [p=d%128, kd*1024 + n]
            for g in range(2):
                pst = ps_big.tile([128, 1024], f32, tag="ps")
                for i in range(4):
                    nch = g * 4 + i
                    nc.tensor.transpose(pst[:, i * 128:(i + 1) * 128],
                                        xsb[:, nch, 0:128], ident)
                    nc.tensor.transpose(pst[:, 512 + i * 128:512 + (i + 1) * 128],
                                        xsb[:, nch, 128:256], ident)
                nc.vector.tensor_copy(
                    xT.rearrange("p (kd g n) -> p kd g n", kd=2, g=2)[:, :, g, :],
                    pst.rearrange("p (kd n) -> p kd n", kd=2))

            # ---- qT, kT (raw; dw applied at exp time) ----
            qT = big.tile([128, 4 * N], bf16)  # [dk + 64*(h%2), (h//2)*1024 + n]
            kT = big.tile([128, 4 * N], bf16)
            for c4 in range(4):
                for nt in range(2):
                    psqk = ps_big.tile([128, 1024], f32, tag="ps")
                    for kd in range(2):
                        nc.tensor.matmul(
                            psqk[:, 0:512],
                            wq_r[:, kd * 512 + c4 * 128:kd * 512 + (c4 + 1) * 128],
                            xT[:, kd * N + nt * 512:kd * N + nt * 512 + 512],
                            start=(kd == 0), stop=(kd == 1))
                    for kd in range(2):
                        nc.tensor.matmul(
                            psqk[:, 512:1024],
                            wk_r[:, kd * 512 + c4 * 128:kd * 512 + (c4 + 1) * 128],
                            xT[:, kd * N + nt * 512:kd * N + nt * 512 + 512],
                            start=(kd == 0), stop=(kd == 1))
                    off = c4 * N + nt * 512
                    nc.vector.tensor_copy(qT[:, off:off + 512], psqk[:, 0:512])
                    nc.vector.tensor_copy(kT[:, off:off + 512], psqk[:, 512:1024])

            # ---- v chunks (bf16) ----
            vaug = [big.tile([128, 512], bf16, name=f"v{t8}") for t8 in range(8)]
            for t8 in range(8):
                psv = ps_big.tile([128, 512], f32, tag="ps")
                for kd in range(2):
                    nc.tensor.matmul(
                        psv, xT[:, kd * N + t8 * 128:kd * N + (t8 + 1) * 128],
                        wv_r[:, kd * 512:(kd + 1) * 512],
                        start=(kd == 0), stop=(kd == 1))
                nc.vector.tensor_copy(vaug[t8], psv)

            # ---- attention ----
            oT = [big.tile([128, 4 * 512], bf16, name=f"oT{st}") for st in range(2)]
            for st in range(2):
                for half in range(2):
                    po = [ps_po.tile([128, 512], f32, tag="po",
                                     name=f"po_{st}_{half}_{a}") for a in range(2)]
                    pd = ps_pd.tile([128, 512], f32, tag="pd")
                    for t8 in range(8):
                        first, last = (t8 == 0), (t8 == 7)
                        exs = []
                        for a in range(2):
                            p_idx = half * 2 + a
                            co = p_idx * N
                            ls = ps_big.tile([128, 1024], f32, tag="ps",
                                             name=f"ls_{st}_{half}_{t8}_{a}")
                            for e in range(2):
                                nc.tensor.matmul(
                                    ls[:, e * 512:(e + 1) * 512], idn,
                                    mTn[t8][:, st * 512:st * 512 + 512],
                                    start=True, stop=False)
                            for e in range(2):
                                nc.tensor.matmul(
                                    ls[:, e * 512:(e + 1) * 512],
                                    kT[e * 64:(e + 1) * 64,
                                       co + t8 * 128:co + (t8 + 1) * 128],
                                    qT[e * 64:(e + 1) * 64,
                                       co + st * 512:co + st * 512 + 512],
                                    start=False, stop=True)
                            ex = expool.tile([128, 1024], bf16, tag="exq")
                            nc.scalar.activation(ex, ls, EXP,
                                                 scale=dwc[:, t8:t8 + 1])
                            exs.append(ex)
                            for e in range(2):
                                h = 2 * p_idx + e
                                nc.tensor.matmul(
                                    po[a][e * 64:(e + 1) * 64, :],
                                    vaug[t8][:, h * 64:(h + 1) * 64],
                                    ex[:, e * 512:(e + 1) * 512],
                                    start=first, stop=last)
                        # 4-way col-packed denominators
                        for a in range(2):
                            for e in range(2):
                                j = 2 * a + e
                                nc.tensor.matmul(
                                    pd[32 * j:32 * j + 1, :], ones_bf,
                                    exs[a][:, e * 512:(e + 1) * 512],
                                    start=first, stop=last,
                                    tile_position=(0, 32 * j))
                    # normalize this half's 4 heads (one folded reciprocal)
                    rden = npool.tile([128, 512], f32, tag="dsb")
                    nc.vector.reciprocal_approx_fast(rden, pd)
                    for a in range(2):
                        for e in range(2):
                            j = 2 * a + e
                            p_idx = half * 2 + a
                            p0 = 64 * e
                            rr = npool.tile([1, 512], f32, tag="rr")
                            nc.sync.dma_start(rr, rden[32 * j:32 * j + 1, :])
                            rb = npool.tile([128, 512], f32, tag="rb")
                            nc.gpsimd.partition_broadcast(rb, rr)
                            nc.vector.tensor_tensor(
                                oT[st][p0:p0 + 64, p_idx * 512:(p_idx + 1) * 512],
                                po[a][p0:p0 + 64, :], rb[p0:p0 + 64, :], MULT)
                # ---- output projection for this st's 4 row-chunks ----
                for s4 in range(4):
                    sc = st * 4 + s4
                    psp = ps_out.tile([128, 256], f32, tag="psp")
                    for cc in range(4):
                        nc.tensor.matmul(
                            psp, oT[st][:, cc * 512 + s4 * 128:cc * 512 + (s4 + 1) * 128],
                            wo_r[:, cc * 256:(cc + 1) * 256],
                            start=(cc == 0), stop=False)
                    nc.tensor.matmul(psp, ones_r[0:1, :], bo_r[0:1, :],
                                     start=False, stop=True)
                    ot = opool.tile([128, 256], f32, tag="outp")
                    nc.vector.tensor_copy(ot, psp)
                    nc.sync.dma_start(out_d[sc * 128:(sc + 1) * 128, :], ot)

    nc.compile()
    return nc


def _get_compiled():
    if 'nc' not in _COMPILED:
        _COMPILED['nc'] = _build()
    return _COMPILED['nc']


def _shard(inputs):
    import ml_dtypes
    x = np.ascontiguousarray(inputs['node_features'], dtype=np.float32)
    em = np.ascontiguousarray(inputs['edge_mask'], dtype=np.float32)
    dw = np.ascontiguousarray(inputs['distance_weights'], dtype=np.float32)
    wq = np.ascontiguousarray(inputs['Wq'], dtype=np.float32)
    wk = np.ascontiguousarray(inputs['Wk'], dtype=np.float32)
    wv = np.ascontiguousarray(inputs['Wv'], dtype=np.float32)
    wo = np.ascontiguousarray(inputs['Wo'], dtype=np.float32)
    bo = np.ascontiguousarray(inputs['bo'], dtype=np.float32).reshape(1, D)
    maps = []
    for b in range(NCORES):
        m_bf = np.ascontiguousarray(1.0 - em[b, 0].T).astype(ml_dtypes.bfloat16)
        maps.append({
            "x": x[b],
            "m": m_bf,
            "dwcol": np.ascontiguousarray(dw[b].reshape(8, 128).T),
            "wq": wq, "wk": wk, "wv": wv, "wo": wo, "bo": bo,
        })
    return maps


def run_sharded(inputs, **kwargs):
    from concourse.bass_utils import run_bass_kernel_spmd
    nc = _get_compiled()
    maps = _shard(inputs)
    res = run_bass_kernel_spmd(nc, maps, core_ids=list(range(NCORES)), **kwargs)
    out = np.stack([res.results[b]["out"] for b in range(NCORES)], axis=0)
    return out, res


def kernel(**inputs) -> np.ndarray:
    out, _ = run_sharded(inputs)
    return out


# revision 24
# speedup vs baseline: 1.1602x; 1.0709x over previous
"""CrystalGraphAttention Trainium2 kernel (v2).

Data-parallel over batch: core b handles batch b (B=8, 8 cores).
Per-core algorithm (transposed layouts, [feature, node]):
  xT = x^T                                  (PE transpose)
  qT = (Wq/8)^T xT, kT = Wk^T xT            (fp32r; head pairs row-packed)
  vaug[tc] = (x Wv)[tc-chunk]               (bf16, plain [128,512] chunks)
  mTn[tc]  = (1 - edge_mask^T) chunk, bf16  (host-prepped, exact 0/1)
  logitsT[t,s] = kT_h^T qT_h + (-1e9 I) @ mTn   (mask folded into the logits
       PSUM by a matmul-accumulate; unmasked entries get exactly +0)
  ex = exp(logits * dw_t)  (ACT, per-partition dw scale; masked -> exp(-1e5)=0)
  po_pair[0:64] += v_he^T ex_he ; po_pair[64:128] += v_ho^T ex_ho (col-packed)
  pd[32j] += ones^T ex_h   (4-way col-packed denominators)
  oT = po / den            (recip + gpsimd partition broadcast)
  out = oT^T Wo + bo       (fp32r, bias via ones-row K=1 accumulation)
"""
import sys

if '/opt/trn_rl_repo' not in sys.path:
    sys.path.insert(0, '/opt/trn_rl_repo')

import os

import numpy as np

B, N, D = 8, 1024, 256
H, DK, DV = 8, 64, 64
NCORES = 8

# Schraudolph exp: exp(x) ~= bitcast_f32(int32(A*x + BC))
A_SCH = float(2.0 ** 23 / np.log(2.0))
B_SCH = float((127.0 - 0.044) * 2.0 ** 23)

_COMPILED = {}


def _build():
    import concourse.bass as bass
    import concourse.mybir as mybir
    import concourse.tile as tile
    from concourse import bacc
    from concourse.masks import make_identity

    f32 = mybir.dt.float32
    f32r = mybir.dt.float32r
    bf16 = mybir.dt.bfloat16
    i32 = mybir.dt.int32
    MULT = mybir.AluOpType.mult
    ADD = mybir.AluOpType.add
    EXP = mybir.ActivationFunctionType.Exp

    n_sch = int(os.environ.get("KSCH", "8"))  # of 32 a==1 pair-tiles

    nc = bacc.Bacc(None, target_bir_lowering=False)

    x_d = nc.dram_tensor("x", [N, D], f32, kind="ExternalInput")
    m_d = nc.dram_tensor("m", [N, N], bf16, kind="ExternalInput")
    dwc_d = nc.dram_tensor("dwcol", [128, 8], f32, kind="ExternalInput")
    wq_d = nc.dram_tensor("wq", [D, H * DK], f32, kind="ExternalInput")
    wk_d = nc.dram_tensor("wk", [D, H * DK], f32, kind="ExternalInput")
    wv_d = nc.dram_tensor("wv", [D, H * DV], f32, kind="ExternalInput")
    wo_d = nc.dram_tensor("wo", [H * DV, D], f32, kind="ExternalInput")
    bo_d = nc.dram_tensor("bo", [1, D], f32, kind="ExternalInput")
    out_d = nc.dram_tensor("out", [N, D], f32, kind="ExternalOutput")

    with tile.TileContext(nc) as tc:
        with tc.tile_pool(name="const", bufs=1) as cst, \
             tc.tile_pool(name="big", bufs=1) as big, \
             tc.tile_pool(name="xst", bufs=2) as xpool, \
             tc.tile_pool(name="wst", bufs=2) as wpool, \
             tc.tile_pool(name="exq", bufs=10) as expool, \
             tc.tile_pool(name="sin", bufs=10) as sinpool, \
             tc.tile_pool(name="nrm", bufs=4) as npool, \
             tc.tile_pool(name="outp", bufs=3) as opool, \
             tc.tile_pool(name="psb", bufs=3, space="PSUM") as ps_big, \
             tc.tile_pool(name="pspo", bufs=2, space="PSUM") as ps_po:

            # ---- constants / tiny inputs ----
            ident = cst.tile([128, 128], f32)
            make_identity(nc, ident)
            idn_f = cst.tile([128, 128], f32)
            nc.vector.tensor_scalar_mul(idn_f, ident, -1.0e9)
            idn = cst.tile([128, 128], bf16)
            nc.vector.tensor_copy(idn, idn_f)

            ones_f = cst.tile([1, 128], f32)
            nc.vector.memset(ones_f, 1.0)
            ones_r = cst.tile([1, 128], bf16)
            nc.vector.tensor_copy(ones_r, ones_f)
            ones_c = cst.tile([128, 1], f32)
            nc.vector.memset(ones_c, 1.0)
            ones_bf = cst.tile([128, 1], bf16)
            nc.vector.tensor_copy(ones_bf, ones_c)

            dwc = cst.tile([128, 8], f32)
            nc.scalar.dma_start(dwc, dwc_d[:, :])
            adw = cst.tile([128, 8], f32)
            nc.vector.tensor_scalar_mul(adw, dwc, A_SCH)

            # ---- bulk input DMAs: x first (scalar queue), mask on sync ----
            xsb = big.tile([128, 8, D], f32, name="xsb")
            nc.scalar.dma_start(xsb, x_d.rearrange("(nch p) j -> p nch j", p=128))

            mTn = [big.tile([128, N], bf16, name=f"mTn{t8}") for t8 in range(8)]
            for t8 in range(8):
                nc.sync.dma_start(mTn[t8], m_d[t8 * 128:(t8 + 1) * 128, :])

            # ---- weights (scalar HWDGE queue) -> fp32r (q pre-scaled 1/8) ----
            def load_w(dram, scale):
                st = wpool.tile([128, 1024], f32, tag="wst")
                nc.scalar.dma_start(
                    st.rearrange("p (kd c) -> p kd c", kd=2),
                    dram.rearrange("(kd p) c -> p kd c", p=128))
                r = big.tile([128, 1024], bf16, name=dram.name + "_r")
                if scale is None:
                    nc.vector.tensor_copy(r, st)
                else:
                    nc.vector.tensor_scalar_mul(r, st, scale)
                return r

            wq_r = load_w(wq_d, 0.125)
            wk_r = load_w(wk_d, None)
            wv_r = load_w(wv_d, None)
            wo_st = wpool.tile([128, 1024], f32, tag="wst")
            nc.scalar.dma_start(
                wo_st.rearrange("p (cc c) -> p cc c", cc=4),
                wo_d.rearrange("(cc p) c -> p cc c", p=128))
            wo_r = big.tile([128, 1024], bf16)
            nc.vector.tensor_copy(wo_r, wo_st)
            bo_f = cst.tile([1, 256], f32)
            nc.scalar.dma_start(bo_f, bo_d[:, :])
            bo_r = cst.tile([1, 256], bf16)
            nc.vector.tensor_copy(bo_r, bo_f)

            # ---- xT via PE transpose ----
            xT = big.tile([128, 2 * N], bf16)  # [p=d%128, kd*1024 + n]
            for g in range(2):
                pst = ps_big.tile([128, 1024], f32, tag="ps")
                for i in range(4):
                    nch = g * 4 + i
                    nc.tensor.transpose(pst[:, i * 128:(i + 1) * 128],
                                        xsb[:, nch, 0:128], ident)
                    nc.tensor.transpose(pst[:, 512 + i * 128:512 + (i + 1) * 128],
                                        xsb[:, nch, 128:256], ident)
                nc.vector.tensor_copy(
                    xT.rearrange("p (kd g n) -> p kd g n", kd=2, g=2)[:, :, g, :],
                    pst.rearrange("p (kd n) -> p kd n", kd=2))

            # ---- qT, kT (raw; dw applied at exp time) ----
            qT = big.tile([128, 4 * N], bf16)  # [dk + 64*(h%2), (h//2)*1024 + n]
            kT = big.tile([128, 4 * N], bf16)
            for c4 in range(4):
                for nt in range(2):
                    psqk = ps_big.tile([128, 1024], f32, tag="ps")
                    for kd in range(2):
                        nc.tensor.matmul(
                            psqk[:, 0:512],
                            wq_r[:, kd * 512 + c4 * 128:kd * 512 + (c4 + 1) * 128],
                            xT[:, kd * N + nt * 512:kd * N + nt * 512 + 512],
                            start=(kd == 0), stop=(kd == 1))
                    for kd in range(2):
                        nc.tensor.matmul(
                            psqk[:, 512:1024],
                            wk_r[:, kd * 512 + c4 * 128:kd * 512 + (c4 + 1) * 128],
                            xT[:, kd * N + nt * 512:kd * N + nt * 512 + 512],
                            start=(kd == 0), stop=(kd == 1))
                    off = c4 * N + nt * 512
                    nc.vector.tensor_copy(qT[:, off:off + 512], psqk[:, 0:512])
                    nc.vector.tensor_copy(kT[:, off:off + 512], psqk[:, 512:1024])

            # ---- v chunks (bf16) ----
            vaug = [big.tile([128, 512], bf16, name=f"v{t8}") for t8 in range(8)]
            for t8 in range(8):
                psv = ps_big.tile([128, 512], f32, tag="ps")
                for kd in range(2):
                    nc.tensor.matmul(
                        psv, xT[:, kd * N + t8 * 128:kd * N + (t8 + 1) * 128],
                        wv_r[:, kd * 512:(kd + 1) * 512],
                        start=(kd == 0), stop=(kd == 1))
                nc.vector.tensor_copy(vaug[t8], psv)

            # ---- attention (pair-major; deferred denominators) ----
            oT = [big.tile([128, 4 * 512], bf16, name=f"oT{st}") for st in range(2)]
            sch_used = 0
            for st in range(2):
                for p_idx in range(4):
                    co = p_idx * N
                    po = ps_po.tile([128, 512], f32, tag="po",
                                    name=f"po_{st}_{p_idx}")
                    exs = []
                    for t8 in range(8):
                        first, last = (t8 == 0), (t8 == 7)
                        ls = ps_big.tile([128, 1024], f32, tag="ps",
                                         name=f"ls_{st}_{p_idx}_{t8}")
                        for e in range(2):
                            nc.tensor.matmul(
                                ls[:, e * 512:(e + 1) * 512], idn,
                                mTn[t8][:, st * 512:st * 512 + 512],
                                start=True, stop=False)
                        for e in range(2):
                            nc.tensor.matmul(
                                ls[:, e * 512:(e + 1) * 512],
                                kT[e * 64:(e + 1) * 64,
                                   co + t8 * 128:co + (t8 + 1) * 128],
                                qT[e * 64:(e + 1) * 64,
                                   co + st * 512:co + st * 512 + 512],
                                start=False, stop=True)
                        use_sch = (sch_used < n_sch) and (t8 % 4 == 2)
                        if use_sch:
                            sch_used += 1
                            sint = sinpool.tile([128, 1024], i32, tag="sin")
                            nc.vector.tensor_scalar(
                                sint, ls, adw[:, t8:t8 + 1], B_SCH, MULT, ADD)
                            ex = sint.bitcast(bf16).rearrange(
                                "p (s two) -> p s two", two=2)[:, :, 1]
                        else:
                            ext = expool.tile([128, 1024], bf16, tag="exq")
                            nc.scalar.activation(ext, ls, EXP,
                                                 scale=dwc[:, t8:t8 + 1])
                            ex = ext
                        exs.append(ex)
                        for e in range(2):
                            h = 2 * p_idx + e
                            nc.tensor.matmul(
                                po[e * 64:(e + 1) * 64, :],
                                vaug[t8][:, h * 64:(h + 1) * 64],
                                ex[:, e * 512:(e + 1) * 512],
                                start=first, stop=last)
                    # deferred denominators: 2-way col-packed burst
                    pd = ps_big.tile([128, 512], f32, tag="ps",
                                     name=f"pd_{st}_{p_idx}")
                    for t8 in range(8):
                        first, last = (t8 == 0), (t8 == 7)
                        for e in range(2):
                            nc.tensor.matmul(
                                pd[64 * e:64 * e + 1, :], ones_bf,
                                exs[t8][:, e * 512:(e + 1) * 512],
                                start=first, stop=last,
                                tile_position=(0, 64 * e))
                    rden = npool.tile([128, 512], f32, tag="dsb")
                    nc.vector.reciprocal_approx_fast(rden, pd)
                    for e in range(2):
                        p0 = 64 * e
                        rr = npool.tile([1, 512], f32, tag="rr")
                        nc.sync.dma_start(rr, rden[64 * e:64 * e + 1, :])
                        rb = npool.tile([128, 512], f32, tag="rb")
                        nc.gpsimd.partition_broadcast(rb, rr)
                        nc.vector.tensor_tensor(
                            oT[st][p0:p0 + 64, p_idx * 512:(p_idx + 1) * 512],
                            po[p0:p0 + 64, :], rb[p0:p0 + 64, :], MULT)
                # ---- output projection for this st's 4 row-chunks ----
                for s4 in range(4):
                    sc = st * 4 + s4
                    psp = ps_big.tile([128, 256], f32, tag="ps",
                                      name=f"psp_{sc}")
                    for cc in range(4):
                        nc.tensor.matmul(
                            psp, oT[st][:, cc * 512 + s4 * 128:cc * 512 + (s4 + 1) * 128],
                            wo_r[:, cc * 256:(cc + 1) * 256],
                            start=(cc == 0), stop=False)
                    nc.tensor.matmul(psp, ones_r[0:1, :], bo_r[0:1, :],
                                     start=False, stop=True)
                    ot = opool.tile([128, 256], f32, tag="outp")
                    nc.vector.tensor_copy(ot, psp)
                    nc.sync.dma_start(out_d[sc * 128:(sc + 1) * 128, :], ot)

    nc.compile()
    return nc


def _get_compiled():
    if 'nc' not in _COMPILED:
        _COMPILED['nc'] = _build()
    return _COMPILED['nc']


def _shard(inputs):
    import ml_dtypes
    x = np.ascontiguousarray(inputs['node_features'], dtype=np.float32)
    em = np.ascontiguousarray(inputs['edge_mask'], dtype=np.float32)
    dw = np.ascontiguousarray(inputs['distance_weights'], dtype=np.float32)
    wq = np.ascontiguousarray(inputs['Wq'], dtype=np.float32)
    wk = np.ascontiguousarray(inputs['Wk'], dtype=np.float32)
    wv = np.ascontiguousarray(inputs['Wv'], dtype=np.float32)
    wo = np.ascontiguousarray(inputs['Wo'], dtype=np.float32)
    bo = np.ascontiguousarray(inputs['bo'], dtype=np.float32).reshape(1, D)
    maps = []
    for b in range(NCORES):
        m_bf = np.ascontiguousarray(1.0 - em[b, 0].T).astype(ml_dtypes.bfloat16)
        maps.append({
            "x": x[b],
            "m": m_bf,
            "dwcol": np.ascontiguousarray(dw[b].reshape(8, 128).T),
            "wq": wq, "wk": wk, "wv": wv, "wo": wo, "bo": bo,
        })
    return maps


def run_sharded(inputs, **kwargs):
    from concourse.bass_utils import run_bass_kernel_spmd
    nc = _get_compiled()
    maps = _shard(inputs)
    res = run_bass_kernel_spmd(nc, maps, core_ids=list(range(NCORES)), **kwargs)
    out = np.stack([res.results[b]["out"] for b in range(NCORES)], axis=0)
    return out, res


def kernel(**inputs) -> np.ndarray:
    out, _ = run_sharded(inputs)
    return out
